# revision 9
# baseline (speedup 1.0000x reference)
"""Trainium2 Bass kernel: 21-layer tiny MLP (2 -> 9 x19 -> 2, ELU, log_softmax)
over batch 2,097,152, data-parallel across 8 NeuronCores.

Layout: within a core (batch shard BC=262144), samples are interleaved into
G=14 block-diagonal groups: sample index = 14*n + g  (g = group, n = column).
Activations live as [126, N] tiles (partition = 9*g + feature), so every
layer is ONE 126x126 block-diagonal matmul per 512 columns on the PE.

ELU(h+b) = relu(h+b) + (min(exp(h+b),1) - 1):
  e = ACT Exp(h + b)         (PSUM -> SBUF, per-partition bias)
  r = DVE TS (h + b) max 0   (PSUM -> SBUF)
  t = DVE TS (e min 1) - 1   (SBUF)
  u = DVE TT t + r           (SBUF)

Final layer computes d = z1 - z0 directly (single output per group);
log_softmax: out0 = -softplus(d+bd) = -ln(1+exp(d+bd)), out1 = (d+bd) - sp.
softplus is built from Exp/Ln (both live in the natural_log_exp table set);
the d values are staged into full-partition SBUF tiles ([14*8, 2048]) so the
tail passes use all partitions.
"""

import numpy as np

B_TOTAL = 2097152
NCORES = 8
BC = B_TOTAL // NCORES  # 262144
G = 14                  # block-diag groups per core
F = 9                   # hidden width
NMID = 19               # fc2..fc20
NLAYERS = NMID + 2      # 21
SC = 2048               # super-chunk columns (4 PSUM banks)
BANDS = 4               # SC bands per stage tile, at 32-partition strides

_BUILD_CACHE = {}
_LAST_RESULTS = {}      # stashed BassKernelResults for test harness introspection


def _plan(bc):
    """Column/tail geometry for a per-core batch of bc samples."""
    ncols = bc // G                 # full columns
    nleft = bc - ncols * G          # leftover samples -> one extra column, groups 0..nleft-1
    n_sc_full = ncols // SC
    rem = ncols - n_sc_full * SC
    # tail SC holds the remaining `rem` full columns plus (if any) the leftover column
    tail_n = rem + (1 if nleft else 0)
    n_sc = n_sc_full + (1 if tail_n else 0)
    return ncols, nleft, n_sc_full, rem, tail_n, n_sc


def _build(bc, sd_name):
    """Build (and cache) the Bass program for per-core batch bc."""
    key = (bc, sd_name)
    if key in _BUILD_CACHE:
        return _BUILD_CACHE[key]

    from contextlib import ExitStack
    import concourse.bacc as bacc
    import concourse.tile as tile
    import concourse.mybir as mybir

    dt = mybir.dt
    AF = mybir.ActivationFunctionType
    OP = mybir.AluOpType

    f32 = dt.float32
    if sd_name == "f32":
        SD = dt.float32      # SBUF activation dtype (e/r/t temps)
        UD = dt.float32r     # u tiles / matmul operand dtype
        XD = dt.float32r     # x input dtype
        WD = dt.float32r     # weights dtype
        MD = dt.float32r     # matmul operand dtype
    else:
        SD = dt.float16
        UD = dt.float16
        XD = dt.float16
        WD = dt.float16
        MD = dt.float16

    ncols, nleft, n_sc_full, rem, tail_n, n_sc = _plan(bc)

    nc = bacc.Bacc("TRN2", target_bir_lowering=False, debug=False)

    x_d = nc.dram_tensor("x", [bc, 2], XD, kind="ExternalInput")
    w_d = nc.dram_tensor("wpack", [128, NLAYERS * 126], WD, kind="ExternalInput")
    b_d = nc.dram_tensor("bpack", [128, NLAYERS], f32, kind="ExternalInput")
    out_d = nc.dram_tensor("out", [bc, 2], f32, kind="ExternalOutput")

    def mm_cast(ap):
        return ap.bitcast(MD) if MD != ap.dtype else ap

    with ExitStack() as ctx:
        tc = ctx.enter_context(tile.TileContext(nc))
        wpool = ctx.enter_context(tc.tile_pool(name="w", bufs=1))
        xpool = ctx.enter_context(tc.tile_pool(name="xin", bufs=3))
        hpool = ctx.enter_context(tc.tile_pool(name="h", bufs=2, space="PSUM"))
        epool = ctx.enter_context(tc.tile_pool(name="e", bufs=2))
        rpool = ctx.enter_context(tc.tile_pool(name="r", bufs=2))
        tpool = ctx.enter_context(tc.tile_pool(name="t", bufs=2))
        upool = ctx.enter_context(tc.tile_pool(name="u", bufs=3))
        spool = ctx.enter_context(tc.tile_pool(name="stage", bufs=1))
        fpool = ctx.enter_context(tc.tile_pool(name="fin", bufs=2))

        wsb = wpool.tile([128, NLAYERS * 126], WD, tag="w")
        nc.sync.dma_start(wsb[:], w_d.ap()[:])
        bsb = wpool.tile([128, NLAYERS], f32, tag="b")
        nc.sync.dma_start(bsb[:], b_d.ap()[:])

        # full-column view of x: partition = 2*g + feature, free = column
        xg = x_d.ap()[0 : G * ncols].rearrange("(n c) f -> (c f) n", c=G)
        xl = None
        if nleft:
            xl = x_d.ap()[G * ncols : bc].rearrange("(n c) f -> (c f) n", c=nleft)

        # stage tiles for the d -> log_softmax tail: one 14-partition band per SC,
        # bands live at 32-partition strides (engine ops need 0/32/64/96 bases)
        n_stage = (n_sc + BANDS - 1) // BANDS
        stages = []
        for st in range(n_stage):
            bands = min(BANDS, n_sc - st * BANDS)
            stg = spool.tile([32 * (bands - 1) + 14, SC], f32, tag=f"st{st}")
            nc.vector.memset(stg[:], 0.0)
            stages.append((stg, bands))

        for s in range(n_sc):
            is_tail = s == n_sc_full and tail_n
            N = SC if not is_tail else tail_n
            # fp32r matmuls require even innermost free sizes -> pad odd tails
            Nm = N + (N % 2)

            xt = xpool.tile([28, SC], XD, tag="x")
            if not is_tail:
                nc.sync.dma_start(xt[:, 0:N], xg[:, s * SC : s * SC + N])
            else:
                if Nm > rem:
                    # leftover/pad columns: only groups 0..nleft-1 get real data
                    pad = xt[:, rem:Nm]
                    if XD == dt.float32r:
                        pad = pad.bitcast(f32)
                    nc.vector.memset(pad, 0.0)
                if rem:
                    nc.sync.dma_start(xt[:, 0:rem], xg[:, n_sc_full * SC : ncols])
                if nleft:
                    nc.sync.dma_start(xt[0 : 2 * nleft, rem : rem + 1], xl)

            cur = xt          # SBUF input of the next matmul
            kin = 28          # partitions of cur
            for layer in range(1, NLAYERS + 1):
                mout = 126 if layer < NLAYERS else G
                lo = (layer - 1) * 126
                h = hpool.tile([126, SC], f32, tag="h")
                lhsT = mm_cast(wsb[0:kin, lo : lo + mout])
                for j in range(0, N, 512):
                    n1 = min(512, N - j)
                    nc.tensor.matmul(
                        h[0:mout, j : j + n1],
                        lhsT=lhsT,
                        rhs=mm_cast(cur[0:kin, j : j + n1]),
                        start=True,
                        stop=True,
                    )
                if layer < NLAYERS:
                    bap = bsb[0:126, layer - 1 : layer]
                    e = epool.tile([126, SC], SD, tag="e")
                    nc.scalar.activation(e[:, 0:N], h[:, 0:N], AF.Exp, bias=bap)
                    r = rpool.tile([126, SC], SD, tag="r")
                    nc.vector.tensor_scalar(
                        r[:, 0:N], h[:, 0:N], bap, 0.0, OP.add, OP.max
                    )
                    t = tpool.tile([126, SC], SD, tag="t")
                    nc.vector.tensor_scalar(
                        t[:, 0:N], e[:, 0:N], 1.0, -1.0, OP.min, OP.add
                    )
                    u = upool.tile([126, SC], UD, tag="u")
                    nc.vector.tensor_tensor(u[:, 0:N], t[:, 0:N], r[:, 0:N], OP.add)
                    cur = u
                    kin = 126
                else:
                    # v = d + bd staged into the band for this SC
                    stg, _ = stages[s // BANDS]
                    band = 32 * (s % BANDS)
                    bdap = bsb[0:G, NLAYERS - 1 : NLAYERS]
                    nc.vector.tensor_scalar_add(
                        stg[band : band + G, 0:N], h[0:G, 0:N], bdap
                    )

        # ---- log_softmax tail over the staged v = d + bd --------------------
        # out0 = -ln(1 + exp(v)); out1 = v - ln(1 + exp(v))
        outs = []
        for stg, bands in stages:
            p = 32 * (bands - 1) + 14
            et = epool.tile([126, SC], f32, tag="e")
            nc.scalar.activation(et[0:p, :], stg[0:p, :], AF.Exp)
            st_ = tpool.tile([126, SC], f32, tag="t")
            nc.vector.tensor_scalar_add(st_[0:p, :], et[0:p, :], 1.0)
            lt = rpool.tile([126, SC], f32, tag="r")
            nc.scalar.activation(lt[0:p, :], st_[0:p, :], AF.Ln)
            o0 = fpool.tile([126, SC], f32, tag="o0")
            nc.vector.tensor_scalar_mul(o0[0:p, :], lt[0:p, :], -1.0)
            o1 = fpool.tile([126, SC], f32, tag="o1")
            nc.vector.tensor_tensor(o1[0:p, :], stg[0:p, :], lt[0:p, :], OP.subtract)
            outs.append((o0, o1))

        # ---- output DMAs (one per band) -------------------------------------
        for st in range(n_stage):
            o0, o1 = outs[st]
            for k in range(stages[st][1]):
                s = st * BANDS + k
                band = 32 * k
                if s < n_sc_full:
                    a = out_d.ap()[s * SC * G : (s + 1) * SC * G].rearrange(
                        "(c g) f -> g c f", g=G
                    )
                    nc.sync.dma_start(a[:, :, 0], o0[band : band + G, :])
                    nc.sync.dma_start(a[:, :, 1], o1[band : band + G, :])
                else:  # tail band
                    if rem:
                        a = out_d.ap()[n_sc_full * SC * G : G * ncols].rearrange(
                            "(c g) f -> g c f", g=G
                        )
                        nc.sync.dma_start(a[:, :, 0], o0[band : band + G, 0:rem])
                        nc.sync.dma_start(a[:, :, 1], o1[band : band + G, 0:rem])
                    if nleft:
                        a = out_d.ap()[G * ncols : bc].rearrange(
                            "(c g) f -> g c f", g=nleft
                        )
                        nc.sync.dma_start(
                            a[:, :, 0], o0[band : band + nleft, rem : rem + 1]
                        )
                        nc.sync.dma_start(
                            a[:, :, 1], o1[band : band + nleft, rem : rem + 1]
                        )

    nc.compile()
    _BUILD_CACHE[key] = nc
    return nc


def _prep_weights(W1, b1, Wmid, bmid, W21, b21, sd_name):
    wdt = np.float32 if sd_name == "f32" else np.float16
    wpack = np.zeros((128, NLAYERS * 126), np.float32)
    for g in range(G):
        wpack[2 * g : 2 * g + 2, 9 * g : 9 * g + 9] = W1.T
    for layer in range(2, NLAYERS):
        Wl = Wmid[layer - 2]
        lo = (layer - 1) * 126
        for g in range(G):
            wpack[9 * g : 9 * g + 9, lo + 9 * g : lo + 9 * g + 9] = Wl.T
    wd = W21[1] - W21[0]
    lo = (NLAYERS - 1) * 126
    for g in range(G):
        wpack[9 * g : 9 * g + 9, lo + g] = wd

    bpack = np.zeros((128, NLAYERS), np.float32)
    bpack[0:126, 0] = np.tile(b1, G)
    for layer in range(2, NLAYERS):
        bpack[0:126, layer - 1] = np.tile(bmid[layer - 2], G)
    bpack[0:G, NLAYERS - 1] = b21[1] - b21[0]
    return wpack.astype(wdt), bpack


def _run(x, W1, b1, Wmid, bmid, W21, b21, sd_name="f32", trace=False):
    from concourse.bass_utils import run_bass_kernel_spmd

    nc = _build(BC, sd_name)
    wpack, bpack = _prep_weights(
        np.asarray(W1, np.float32),
        np.asarray(b1, np.float32),
        np.asarray(Wmid, np.float32),
        np.asarray(bmid, np.float32),
        np.asarray(W21, np.float32),
        np.asarray(b21, np.float32),
        sd_name,
    )
    x = np.asarray(x)
    xdt = np.float32 if sd_name == "f32" else np.float16
    in_maps = []
    for c in range(NCORES):
        xs = np.ascontiguousarray(x[c * BC : (c + 1) * BC]).astype(xdt)
        in_maps.append({"x": xs, "wpack": wpack, "bpack": bpack})
    res = run_bass_kernel_spmd(
        nc, in_maps, core_ids=list(range(NCORES)), trace=trace
    )
    _LAST_RESULTS["res"] = res
    out = np.concatenate([r["out"] for r in res.results], axis=0)
    return np.ascontiguousarray(out.astype(np.float32))


def kernel(x, W1, b1, Wmid, bmid, W21, b21):
    return _run(x, W1, b1, Wmid, bmid, W21, b21, sd_name="f32")


# revision 14
# speedup vs baseline: 829.5909x; 829.5909x over previous
"""Trainium2 Bass kernel: 21-layer tiny MLP (2 -> 9 x19 -> 2, ELU, log_softmax)
over batch 2,097,152, data-parallel across 8 NeuronCores.

Layout: within a core (batch shard BC=262144), samples are interleaved into
G=14 block-diagonal groups: sample index = 14*n + g  (g = group, n = column).
Activations live as [126, N] tiles (partition = 9*g + feature), so every
layer is ONE 126x126 block-diagonal matmul per <=512 columns on the PE.

The input is re-packed on the host to the feature-major layout [28, ncols+1]
(row = 2*g + feature, contiguous columns; leftover samples in the last
column), so all device DMAs are contiguous. Outputs are written raw in the
kernel's staged layout and re-assembled on the host.

ELU(h+b) = max(h + b, min(exp(h+b), 1) - 1):
  e = ACT Exp(h + b)                          (PSUM -> SBUF, per-partition bias)
  u = DVE fused-custom max(h + b, min(e,1)-1) (one pass; stock 3-pass fallback)

Final layer computes d = z1 - z0 directly (single output per group);
log_softmax: out0 = -softplus(d+bd) = -ln(1+exp(d+bd)), out1 = (d+bd) - sp,
with d staged into 4-band (32-partition-stride) SBUF tiles so the softplus
tail runs on >100 partitions. Exp and Ln share one ACT table set.
"""

import numpy as np

B_TOTAL = 2097152
NCORES = 8
BC = B_TOTAL // NCORES  # 262144
G = 14                  # block-diag groups per core
F = 9                   # hidden width
NMID = 19               # fc2..fc20
NLAYERS = NMID + 2      # 21
SC = 1024               # super-chunk columns (2 PSUM banks -> 4 h slots)
BANDS = 4               # SC bands per stage tile, at 32-partition strides
PBAND = 32 * (BANDS - 1) + 14  # 110

_BUILD_CACHE = {}
_LAST_RESULTS = {}      # stashed results for test harness introspection


def _plan(bc):
    ncols = bc // G                 # full columns
    nleft = bc - ncols * G          # leftovers -> one extra column (groups 0..nleft-1)
    n_sc_full = ncols // SC
    rem = ncols - n_sc_full * SC
    tail_n = rem + (1 if nleft else 0)
    n_sc = n_sc_full + (1 if tail_n else 0)
    n_stage = (n_sc + BANDS - 1) // BANDS
    xcols = ncols + (1 if nleft else 0)
    return ncols, nleft, n_sc_full, rem, tail_n, n_sc, n_stage, xcols


def _register_elu_op():
    """Fused custom DVE op: out = max(in0 + s0, min(in1, 1) - 1).

    in0 = pre-activation h (PSUM fp32), s0 = per-partition bias, in1 = exp(h+b)
    (SBUF). One DVE pass replaces the r/t/u three-pass ELU combine.
    """
    from concourse import dve_ops
    from concourse.dve_spec import Spec, Src0, Src1, C0, One, maxx, minn, lower

    name = "ELU_FUSE_ANT"
    for o in dve_ops.OPS:
        if o.name == name:
            return o
    from concourse.dve_uop import DveOpSpec

    body = maxx(Src0 + C0, minn(Src1, One) - One)

    def ref(in0, in1, s0, s1, imm2):
        return np.maximum(
            in0.astype(np.float32) + s0, np.minimum(in1, 1.0) - 1.0
        ).astype(np.float32)

    spec = Spec(body=body, reference=ref)
    row = dve_ops._CUSTOM_DVE_ROW_BASE + len(dve_ops.OPS)
    assert row < 0x20
    shas = {}
    for ver in ("v3", "v4"):
        tmp = DveOpSpec(name=name, opcode=row, uops=lower(spec, ver=ver), rd1_en=True)
        shas[ver] = tmp.sha(ver)
    op = dve_ops.DveOp(name, spec, subdim=False, uops_sha=shas)
    dve_ops.OPS.append(op)
    dve_ops._SUB_OPCODE_FOR_NAME[name] = row
    dve_ops.CUSTOM_DVE_SPECS[name] = spec
    return op


def _build(bc, sd_name):
    key = (bc, sd_name)
    if key in _BUILD_CACHE:
        return _BUILD_CACHE[key]

    from contextlib import ExitStack
    import concourse.bacc as bacc
    import concourse.tile as tile
    import concourse.mybir as mybir

    dt = mybir.dt
    AF = mybir.ActivationFunctionType
    OP = mybir.AluOpType

    f32 = dt.float32
    fused = sd_name.endswith("c")
    if sd_name.startswith("f32"):
        SD = dt.float32      # SBUF e-tile dtype
        UD = dt.float32r     # u tiles / matmul operand dtype
        XD = dt.float32r
        WD = dt.float32r
        MD = dt.float32r
    else:
        SD = dt.float16
        UD = dt.float16
        XD = dt.float16
        WD = dt.float16
        MD = dt.float16
    elu_op = _register_elu_op() if fused else None

    ncols, nleft, n_sc_full, rem, tail_n, n_sc, n_stage, xcols = _plan(bc)

    nc = bacc.Bacc("TRN2", target_bir_lowering=False, debug=False)

    x_d = nc.dram_tensor("xp", [28, xcols], XD, kind="ExternalInput")
    w_d = nc.dram_tensor("wpack", [128, NLAYERS * 126], WD, kind="ExternalInput")
    b_d = nc.dram_tensor("bpack", [128, NLAYERS], f32, kind="ExternalInput")
    o0_d = nc.dram_tensor("o0", [n_stage, PBAND, SC], f32, kind="ExternalOutput")
    o1_d = nc.dram_tensor("o1", [n_stage, PBAND, SC], f32, kind="ExternalOutput")

    with ExitStack() as ctx:
        tc = ctx.enter_context(tile.TileContext(nc))
        wpool = ctx.enter_context(tc.tile_pool(name="w", bufs=1))
        xpool = ctx.enter_context(tc.tile_pool(name="xin", bufs=4))
        hpool = ctx.enter_context(tc.tile_pool(name="h", bufs=4, space="PSUM"))
        epool = ctx.enter_context(tc.tile_pool(name="e", bufs=4))
        upool = ctx.enter_context(tc.tile_pool(name="u", bufs=4))
        rpool = ctx.enter_context(tc.tile_pool(name="r", bufs=3))
        tpool = ctx.enter_context(tc.tile_pool(name="t", bufs=3))
        spool = ctx.enter_context(tc.tile_pool(name="stage", bufs=1))
        fpool = ctx.enter_context(tc.tile_pool(name="fin", bufs=2))

        wsb = wpool.tile([128, NLAYERS * 126], WD, tag="w")
        nc.sync.dma_start(wsb[:], w_d.ap()[:])
        bsb = wpool.tile([128, NLAYERS], f32, tag="b")
        nc.sync.dma_start(bsb[:], b_d.ap()[:])

        xg = x_d.ap()

        stages = []
        for st in range(n_stage):
            bands = min(BANDS, n_sc - st * BANDS)
            stg = spool.tile([32 * (bands - 1) + 14, SC], f32, tag=f"st{st}")
            nc.vector.memset(stg[:], 0.0)
            stages.append((stg, bands))

        for s in range(n_sc):
            is_tail = s == n_sc_full and tail_n
            N = SC if not is_tail else tail_n
            Nm = N + (N % 2)  # fp32r matmuls need even innermost free sizes

            xt = xpool.tile([28, SC], XD, tag="x")
            nc.sync.dma_start(xt[:, 0:N], xg[:, s * SC : s * SC + N])
            if Nm > N:
                pad = xt[:, N:Nm]
                if XD == dt.float32r:
                    pad = pad.bitcast(f32)
                nc.vector.memset(pad, 0.0)

            cur = xt
            kin = 28
            for layer in range(1, NLAYERS + 1):
                mout = 126 if layer < NLAYERS else G
                lo = (layer - 1) * 126
                h = hpool.tile([126, SC], f32, tag="h")
                lhsT = wsb[0:kin, lo : lo + mout]
                for j in range(0, Nm, 512):
                    n1 = min(512, Nm - j)
                    nc.tensor.matmul(
                        h[0:mout, j : j + n1],
                        lhsT=lhsT,
                        rhs=cur[0:kin, j : j + n1],
                        start=True,
                        stop=True,
                    )
                if layer < NLAYERS:
                    bap = bsb[0:126, layer - 1 : layer]
                    e = epool.tile([126, SC], SD, tag="e")
                    nc.scalar.activation(e[:, 0:Nm], h[:, 0:Nm], AF.Exp, bias=bap)
                    u = upool.tile([126, SC], UD, tag="u")
                    if fused:
                        nc.vector._custom_dve(
                            elu_op, out=u[:, 0:Nm], in0=h[:, 0:Nm],
                            in1=e[:, 0:Nm], s0=bap,
                        )
                    else:
                        r = rpool.tile([126, SC], SD, tag="r")
                        nc.vector.tensor_scalar(
                            r[:, 0:Nm], h[:, 0:Nm], bap, 0.0, OP.add, OP.max
                        )
                        t = tpool.tile([126, SC], SD, tag="t")
                        nc.vector.tensor_scalar(
                            t[:, 0:Nm], e[:, 0:Nm], 1.0, -1.0, OP.min, OP.add
                        )
                        nc.vector.tensor_tensor(
                            u[:, 0:Nm], t[:, 0:Nm], r[:, 0:Nm], OP.add
                        )
                    cur = u
                    kin = 126
                else:
                    stg, _ = stages[s // BANDS]
                    band = 32 * (s % BANDS)
                    bdap = bsb[0:G, NLAYERS - 1 : NLAYERS]
                    nc.vector.tensor_scalar_add(
                        stg[band : band + G, 0:Nm], h[0:G, 0:Nm], bdap
                    )

        # ---- log_softmax tail over the staged v = d + bd --------------------
        # out0 = -ln(1 + exp(v)); out1 = v - ln(1 + exp(v))
        for st, (stg, bands) in enumerate(stages):
            p = 32 * (bands - 1) + 14
            et = epool.tile([126, SC], f32, tag="e")
            nc.scalar.activation(et[0:p, :], stg[0:p, :], AF.Exp)
            st_ = tpool.tile([126, SC], f32, tag="t")
            nc.vector.tensor_scalar_add(st_[0:p, :], et[0:p, :], 1.0)
            lt = rpool.tile([126, SC], f32, tag="r")
            nc.scalar.activation(lt[0:p, :], st_[0:p, :], AF.Ln)
            o0 = fpool.tile([126, SC], f32, tag="o0")
            nc.vector.tensor_scalar_mul(o0[0:p, :], lt[0:p, :], -1.0)
            o1 = fpool.tile([126, SC], f32, tag="o1")
            nc.vector.tensor_tensor(o1[0:p, :], stg[0:p, :], lt[0:p, :], OP.subtract)
            nc.sync.dma_start(o0_d.ap()[st, 0:p, :], o0[0:p, :])
            nc.sync.dma_start(o1_d.ap()[st, 0:p, :], o1[0:p, :])

    nc.compile()
    _BUILD_CACHE[key] = nc
    return nc


def _prep_weights(W1, b1, Wmid, bmid, W21, b21, sd_name):
    wdt = np.float32 if sd_name.startswith("f32") else np.float16
    wpack = np.zeros((128, NLAYERS * 126), np.float32)
    for g in range(G):
        wpack[2 * g : 2 * g + 2, 9 * g : 9 * g + 9] = W1.T
    for layer in range(2, NLAYERS):
        Wl = Wmid[layer - 2]
        lo = (layer - 1) * 126
        for g in range(G):
            wpack[9 * g : 9 * g + 9, lo + 9 * g : lo + 9 * g + 9] = Wl.T
    wd = W21[1] - W21[0]
    lo = (NLAYERS - 1) * 126
    for g in range(G):
        wpack[9 * g : 9 * g + 9, lo + g] = wd

    bpack = np.zeros((128, NLAYERS), np.float32)
    bpack[0:126, 0] = np.tile(b1, G)
    for layer in range(2, NLAYERS):
        bpack[0:126, layer - 1] = np.tile(bmid[layer - 2], G)
    bpack[0:G, NLAYERS - 1] = b21[1] - b21[0]
    return wpack.astype(wdt), bpack


def _prep_x(xs, bc, xdt):
    """[bc, 2] -> feature-major [28, ncols(+1)] (row = 2g+f, col = n)."""
    ncols, nleft, _, _, _, _, _, xcols = _plan(bc)
    xp = np.zeros((28, xcols), np.float32)
    main = xs[: ncols * G].reshape(ncols, G, 2).transpose(1, 2, 0).reshape(28, ncols)
    xp[:, 0:ncols] = main
    if nleft:
        xp[0 : 2 * nleft, ncols] = xs[ncols * G :].reshape(nleft * 2)
    return np.ascontiguousarray(xp.astype(xdt))


def _unpack_out(o0_raw, o1_raw, bc):
    """staged [n_stage, PBAND, SC] pair -> [bc, 2]."""
    ncols, nleft, _, _, _, n_sc, n_stage, _ = _plan(bc)
    idx = (np.arange(BANDS)[:, None] * 32 + np.arange(G)).reshape(-1)  # [56]
    out = np.empty((bc, 2), np.float32)
    for j, raw in enumerate((o0_raw, o1_raw)):
        sel = raw[:, idx, :].reshape(n_stage, BANDS, G, SC)
        cols = sel.transpose(0, 1, 3, 2).reshape(n_stage * BANDS * SC, G)
        out[: ncols * G, j] = cols[:ncols].reshape(-1)
        if nleft:
            out[ncols * G :, j] = cols[ncols, :nleft]
    return out


def _in_maps(x, W1, b1, Wmid, bmid, W21, b21, sd_name):
    wpack, bpack = _prep_weights(
        np.asarray(W1, np.float32), np.asarray(b1, np.float32),
        np.asarray(Wmid, np.float32), np.asarray(bmid, np.float32),
        np.asarray(W21, np.float32), np.asarray(b21, np.float32), sd_name,
    )
    x = np.asarray(x)
    xdt = np.float32 if sd_name.startswith("f32") else np.float16
    maps = []
    for c in range(NCORES):
        xp = _prep_x(x[c * BC : (c + 1) * BC], BC, xdt)
        maps.append({"xp": xp, "wpack": wpack, "bpack": bpack})
    return maps


def _run(x, W1, b1, Wmid, bmid, W21, b21, sd_name="f16c", trace=False):
    from concourse.bass_utils import run_bass_kernel_spmd

    nc = _build(BC, sd_name)
    in_maps = _in_maps(x, W1, b1, Wmid, bmid, W21, b21, sd_name)
    res = run_bass_kernel_spmd(
        nc, in_maps, core_ids=list(range(NCORES)), trace=trace
    )
    _LAST_RESULTS["res"] = res
    outs = [
        _unpack_out(r["o0"], r["o1"], BC) for r in res.results
    ]
    return np.ascontiguousarray(np.concatenate(outs, axis=0))


def kernel(x, W1, b1, Wmid, bmid, W21, b21):
    return _run(x, W1, b1, Wmid, bmid, W21, b21, sd_name="f16c")


# revision 18
# speedup vs baseline: 1529.5012x; 1.8437x over previous
"""Trainium2 Bass kernel: 21-layer tiny MLP (2 -> 9 x19 -> 2, ELU, log_softmax)
over batch 2,097,152, data-parallel across 8 NeuronCores.

Layout: within a core (batch shard BC=262144), samples are interleaved into
G=14 block-diagonal groups: sample index = 14*n + g  (g = group, n = column).
Activations live as [126, N] tiles (partition = 9*g + feature), so every
layer is ONE 126x126 block-diagonal matmul per <=512 columns on the PE.

The input is re-packed on the host to the feature-major layout [28, ncols+1]
(row = 2*g + feature, contiguous columns; leftover samples in the last
column), so all device DMAs are contiguous. Outputs are written raw in the
kernel's staged layout and re-assembled on the host.

ELU(h+b) = max(h + b, min(exp(h+b), 1) - 1):
  e = ACT Exp(h + b)                          (PSUM -> SBUF, per-partition bias)
  u = DVE fused-custom max(h + b, min(e,1)-1) (one pass; stock 3-pass fallback)

Final layer computes d = z1 - z0 directly (single output per group);
log_softmax: out0 = -softplus(d+bd) = -ln(1+exp(d+bd)), out1 = (d+bd) - sp,
with d staged into 4-band (32-partition-stride) SBUF tiles so the softplus
tail runs on >100 partitions. Exp and Ln share one ACT table set.
"""

import numpy as np

B_TOTAL = 2097152
NCORES = 8
BC = B_TOTAL // NCORES  # 262144
G = 14                  # block-diag groups per core
F = 9                   # hidden width
NMID = 19               # fc2..fc20
NLAYERS = NMID + 2      # 21
SC = 1024               # super-chunk columns (2 PSUM banks per h tile)
NS = 5                  # interleaved SC streams
HBUFS = 4
EBUFS = 8
UBUFS = 8
XBUFS = 6
BANDS = 4               # SC bands per stage tile, at 32-partition strides
PBAND = 32 * (BANDS - 1) + 14  # 110

_BUILD_CACHE = {}
_LAST_RESULTS = {}      # stashed results for test harness introspection


def _plan(bc):
    ncols = bc // G                 # full columns
    nleft = bc - ncols * G          # leftovers -> one extra column (groups 0..nleft-1)
    n_sc_full = ncols // SC
    rem = ncols - n_sc_full * SC
    tail_n = rem + (1 if nleft else 0)
    n_sc = n_sc_full + (1 if tail_n else 0)
    n_stage = (n_sc + BANDS - 1) // BANDS
    xcols = ncols + (1 if nleft else 0)
    return ncols, nleft, n_sc_full, rem, tail_n, n_sc, n_stage, xcols


def _register_elu_op():
    """Fused custom DVE op: out = max(in0 + s0, min(in1, 1) - 1).

    in0 = pre-activation h (PSUM fp32), s0 = per-partition bias, in1 = exp(h+b)
    (SBUF). One DVE pass replaces the r/t/u three-pass ELU combine.
    """
    from concourse import dve_ops
    from concourse.dve_spec import Spec, Src0, Src1, C0, One, maxx, minn, lower

    name = "ELU_FUSE_ANT"
    for o in dve_ops.OPS:
        if o.name == name:
            return o
    from concourse.dve_uop import DveOpSpec

    body = maxx(Src0 + C0, minn(Src1, One) - One)

    def ref(in0, in1, s0, s1, imm2):
        return np.maximum(
            in0.astype(np.float32) + s0, np.minimum(in1, 1.0) - 1.0
        ).astype(np.float32)

    spec = Spec(body=body, reference=ref)
    row = dve_ops._CUSTOM_DVE_ROW_BASE + len(dve_ops.OPS)
    assert row < 0x20
    shas = {}
    for ver in ("v3", "v4"):
        tmp = DveOpSpec(name=name, opcode=row, uops=lower(spec, ver=ver), rd1_en=True)
        shas[ver] = tmp.sha(ver)
    op = dve_ops.DveOp(name, spec, subdim=False, uops_sha=shas)
    dve_ops.OPS.append(op)
    dve_ops._SUB_OPCODE_FOR_NAME[name] = row
    dve_ops.CUSTOM_DVE_SPECS[name] = spec
    return op


def _build(bc, sd_name):
    key = (bc, sd_name)
    if key in _BUILD_CACHE:
        return _BUILD_CACHE[key]

    from contextlib import ExitStack
    import concourse.bacc as bacc
    import concourse.tile as tile
    import concourse.mybir as mybir

    dt = mybir.dt
    AF = mybir.ActivationFunctionType
    OP = mybir.AluOpType

    f32 = dt.float32
    fused = sd_name.endswith("c")
    if sd_name.startswith("f32"):
        SD = dt.float32      # SBUF e-tile dtype
        UD = dt.float32r     # u tiles / matmul operand dtype
        XD = dt.float32r
        WD = dt.float32r
        MD = dt.float32r
    else:
        SD = dt.float16
        UD = dt.float16
        XD = dt.float16
        WD = dt.float16
        MD = dt.float16
    elu_op = _register_elu_op() if fused else None

    ncols, nleft, n_sc_full, rem, tail_n, n_sc, n_stage, xcols = _plan(bc)

    nc = bacc.Bacc("TRN2", target_bir_lowering=False, debug=False)

    x_d = nc.dram_tensor("xp", [28, xcols], XD, kind="ExternalInput")
    w_d = nc.dram_tensor("wpack", [128, NLAYERS * 126], WD, kind="ExternalInput")
    b_d = nc.dram_tensor("bpack", [128, NLAYERS], f32, kind="ExternalInput")
    o0_d = nc.dram_tensor("o0", [n_stage, PBAND, SC], f32, kind="ExternalOutput")
    o1_d = nc.dram_tensor("o1", [n_stage, PBAND, SC], f32, kind="ExternalOutput")

    with ExitStack() as ctx:
        tc = ctx.enter_context(tile.TileContext(nc))
        wpool = ctx.enter_context(tc.tile_pool(name="w", bufs=1))
        xpool = ctx.enter_context(tc.tile_pool(name="xin", bufs=XBUFS))
        hpool = ctx.enter_context(tc.tile_pool(name="h", bufs=HBUFS, space="PSUM"))
        epool = ctx.enter_context(tc.tile_pool(name="e", bufs=EBUFS))
        upool = ctx.enter_context(tc.tile_pool(name="u", bufs=UBUFS))
        rpool = ctx.enter_context(tc.tile_pool(name="r", bufs=3))
        tpool = ctx.enter_context(tc.tile_pool(name="t", bufs=3))
        spool = ctx.enter_context(tc.tile_pool(name="stage", bufs=1))
        fpool = ctx.enter_context(tc.tile_pool(name="fin", bufs=2))

        wsb = wpool.tile([128, NLAYERS * 126], WD, tag="w")
        nc.sync.dma_start(wsb[:], w_d.ap()[:])
        bsb = wpool.tile([128, NLAYERS], f32, tag="b")
        nc.sync.dma_start(bsb[:], b_d.ap()[:])

        xg = x_d.ap()

        stages = []
        for st in range(n_stage):
            bands = min(BANDS, n_sc - st * BANDS)
            stg = spool.tile([32 * (bands - 1) + 14, SC], f32, tag=f"st{st}")
            nc.vector.memset(stg[:], 0.0)
            stages.append((stg, bands))

        for s0 in range(0, n_sc, NS):
            group = list(range(s0, min(s0 + NS, n_sc)))
            state = {}
            for s in group:
                is_tail = s == n_sc_full and tail_n
                N = SC if not is_tail else tail_n
                Nm = N + (N % 2)  # fp32r matmuls need even innermost sizes

                xt = xpool.tile([28, SC], XD, tag="x")
                nc.sync.dma_start(xt[:, 0:N], xg[:, s * SC : s * SC + N])
                if Nm > N:
                    pad = xt[:, N:Nm]
                    if XD == dt.float32r:
                        pad = pad.bitcast(f32)
                    nc.vector.memset(pad, 0.0)
                state[s] = (xt, 28, Nm)

            for layer in range(1, NLAYERS + 1):
                mout = 126 if layer < NLAYERS else G
                lo = (layer - 1) * 126
                lhsT_full = wsb[:, lo : lo + mout]
                for s in group:
                    cur, kin, Nm = state[s]
                    h = hpool.tile([126, SC], f32, tag="h")
                    lhsT = lhsT_full[0:kin]
                    for j in range(0, Nm, 512):
                        n1 = min(512, Nm - j)
                        nc.tensor.matmul(
                            h[0:mout, j : j + n1],
                            lhsT=lhsT,
                            rhs=cur[0:kin, j : j + n1],
                            start=True,
                            stop=True,
                        )
                    if layer < NLAYERS:
                        bap = bsb[0:126, layer - 1 : layer]
                        e = epool.tile([126, SC], SD, tag="e")
                        nc.scalar.activation(
                            e[:, 0:Nm], h[:, 0:Nm], AF.Exp, bias=bap
                        )
                        u = upool.tile([126, SC], UD, tag="u")
                        if fused:
                            nc.vector._custom_dve(
                                elu_op, out=u[:, 0:Nm], in0=h[:, 0:Nm],
                                in1=e[:, 0:Nm], s0=bap,
                            )
                        else:
                            r = rpool.tile([126, SC], SD, tag="r")
                            nc.vector.tensor_scalar(
                                r[:, 0:Nm], h[:, 0:Nm], bap, 0.0, OP.add, OP.max
                            )
                            t = tpool.tile([126, SC], SD, tag="t")
                            nc.vector.tensor_scalar(
                                t[:, 0:Nm], e[:, 0:Nm], 1.0, -1.0, OP.min, OP.add
                            )
                            nc.vector.tensor_tensor(
                                u[:, 0:Nm], t[:, 0:Nm], r[:, 0:Nm], OP.add
                            )
                        state[s] = (u, 126, Nm)
                    else:
                        stg, _ = stages[s // BANDS]
                        band = 32 * (s % BANDS)
                        bdap = bsb[0:G, NLAYERS - 1 : NLAYERS]
                        nc.vector.tensor_scalar_add(
                            stg[band : band + G, 0:Nm], h[0:G, 0:Nm], bdap
                        )

        # ---- log_softmax tail over the staged v = d + bd --------------------
        # out0 = -ln(1 + exp(v)); out1 = v - ln(1 + exp(v))
        for st, (stg, bands) in enumerate(stages):
            p = 32 * (bands - 1) + 14
            et = epool.tile([126, SC], f32, tag="e")
            nc.scalar.activation(et[0:p, :], stg[0:p, :], AF.Exp)
            st_ = tpool.tile([126, SC], f32, tag="t")
            nc.vector.tensor_scalar_add(st_[0:p, :], et[0:p, :], 1.0)
            lt = rpool.tile([126, SC], f32, tag="r")
            nc.scalar.activation(lt[0:p, :], st_[0:p, :], AF.Ln)
            o0 = fpool.tile([126, SC], f32, tag="o0")
            nc.vector.tensor_scalar_mul(o0[0:p, :], lt[0:p, :], -1.0)
            o1 = fpool.tile([126, SC], f32, tag="o1")
            nc.vector.tensor_tensor(o1[0:p, :], stg[0:p, :], lt[0:p, :], OP.subtract)
            nc.sync.dma_start(o0_d.ap()[st, 0:p, :], o0[0:p, :])
            nc.sync.dma_start(o1_d.ap()[st, 0:p, :], o1[0:p, :])

    nc.compile()
    _BUILD_CACHE[key] = nc
    return nc


def _prep_weights(W1, b1, Wmid, bmid, W21, b21, sd_name):
    wdt = np.float32 if sd_name.startswith("f32") else np.float16
    wpack = np.zeros((128, NLAYERS * 126), np.float32)
    for g in range(G):
        wpack[2 * g : 2 * g + 2, 9 * g : 9 * g + 9] = W1.T
    for layer in range(2, NLAYERS):
        Wl = Wmid[layer - 2]
        lo = (layer - 1) * 126
        for g in range(G):
            wpack[9 * g : 9 * g + 9, lo + 9 * g : lo + 9 * g + 9] = Wl.T
    wd = W21[1] - W21[0]
    lo = (NLAYERS - 1) * 126
    for g in range(G):
        wpack[9 * g : 9 * g + 9, lo + g] = wd

    bpack = np.zeros((128, NLAYERS), np.float32)
    bpack[0:126, 0] = np.tile(b1, G)
    for layer in range(2, NLAYERS):
        bpack[0:126, layer - 1] = np.tile(bmid[layer - 2], G)
    bpack[0:G, NLAYERS - 1] = b21[1] - b21[0]
    return wpack.astype(wdt), bpack


def _prep_x(xs, bc, xdt):
    """[bc, 2] -> feature-major [28, ncols(+1)] (row = 2g+f, col = n)."""
    ncols, nleft, _, _, _, _, _, xcols = _plan(bc)
    xp = np.zeros((28, xcols), np.float32)
    main = xs[: ncols * G].reshape(ncols, G, 2).transpose(1, 2, 0).reshape(28, ncols)
    xp[:, 0:ncols] = main
    if nleft:
        xp[0 : 2 * nleft, ncols] = xs[ncols * G :].reshape(nleft * 2)
    return np.ascontiguousarray(xp.astype(xdt))


def _unpack_out(o0_raw, o1_raw, bc):
    """staged [n_stage, PBAND, SC] pair -> [bc, 2]."""
    ncols, nleft, _, _, _, n_sc, n_stage, _ = _plan(bc)
    idx = (np.arange(BANDS)[:, None] * 32 + np.arange(G)).reshape(-1)  # [56]
    out = np.empty((bc, 2), np.float32)
    for j, raw in enumerate((o0_raw, o1_raw)):
        sel = raw[:, idx, :].reshape(n_stage, BANDS, G, SC)
        cols = sel.transpose(0, 1, 3, 2).reshape(n_stage * BANDS * SC, G)
        out[: ncols * G, j] = cols[:ncols].reshape(-1)
        if nleft:
            out[ncols * G :, j] = cols[ncols, :nleft]
    return out


def _in_maps(x, W1, b1, Wmid, bmid, W21, b21, sd_name):
    wpack, bpack = _prep_weights(
        np.asarray(W1, np.float32), np.asarray(b1, np.float32),
        np.asarray(Wmid, np.float32), np.asarray(bmid, np.float32),
        np.asarray(W21, np.float32), np.asarray(b21, np.float32), sd_name,
    )
    x = np.asarray(x)
    xdt = np.float32 if sd_name.startswith("f32") else np.float16
    maps = []
    for c in range(NCORES):
        xp = _prep_x(x[c * BC : (c + 1) * BC], BC, xdt)
        maps.append({"xp": xp, "wpack": wpack, "bpack": bpack})
    return maps


def _run(x, W1, b1, Wmid, bmid, W21, b21, sd_name="f16c", trace=False):
    from concourse.bass_utils import run_bass_kernel_spmd

    nc = _build(BC, sd_name)
    in_maps = _in_maps(x, W1, b1, Wmid, bmid, W21, b21, sd_name)
    res = run_bass_kernel_spmd(
        nc, in_maps, core_ids=list(range(NCORES)), trace=trace
    )
    _LAST_RESULTS["res"] = res
    outs = [
        _unpack_out(r["o0"], r["o1"], BC) for r in res.results
    ]
    return np.ascontiguousarray(np.concatenate(outs, axis=0))


def kernel(x, W1, b1, Wmid, bmid, W21, b21):
    return _run(x, W1, b1, Wmid, bmid, W21, b21, sd_name="f16c")
